# revision 26
# baseline (speedup 1.0000x reference)
"""Trainium2 Bass kernel for nn_MultiHeadAttention (B=2, S=2048, D=1024, H=16, dk=dv=64).

Sharding: 8 cores = 2 batches x 4 head-groups (4 heads per core, data+tensor parallel).
Each core computes, for its (batch b, heads h0..h0+3):
  - Q/K projections in transposed layout Qt/Kt [dk, S] (+1 augmented mask row),
    fp32r matmuls
  - V projection in natural layout [S, dv] (+1 appended ones column for row-sums),
    stored bf16
  - scores St[j, i] = Kaug^T Qaug; masking happens inside the matmul via the
    augmented contraction row: score += cm[j] * rvalid[i]
  - E = exp(St) -> bf16 (no max subtraction needed: |scores| <= ~15, and
    fully-masked rows come out as exp(0)=1 -> uniform 1/S, exactly matching the
    reference)
  - O'^T = Vaug^T E accumulated on PSUM; row 64 = softmax denominators r
  - 1/r computed as exp(-ln r) on ACT (DVE reciprocal is slow, ACT Reciprocal
    is gated), broadcast across partitions with a K=1 fp32 matmul
  - attn written as A^T = E * (1/r) in bf16 (host transposes + upcasts)
  - out partial = (O^T/r)^T @ Wfc rows for this head group (host sums groups)
"""

import math

import numpy as np
import ml_dtypes

import concourse.bass as bass
import concourse.mybir as mybir
from concourse.tile import TileContext
from concourse.vector_clock import ScopedClock
from concourse.bass_utils import run_bass_kernel_spmd

B, S, D = 2, 2048, 1024
H, DK, DV = 16, 64, 64
HPC = 4          # heads per core
NCORES = 8
NEG = -1e30
SCALE = math.log(S) / math.log(512.0) / math.sqrt(DK)

F32 = mybir.dt.float32
F32R = mybir.dt.float32r
BF16 = mybir.dt.bfloat16
F16 = mybir.dt.float16
AF = mybir.ActivationFunctionType
MULT = mybir.AluOpType.mult

SJT = S // 128    # 16 j-tiles
NIC = 4           # i-chunks
IC = S // NIC     # 512 wide


# --- workaround: this walrus build accepts only ONE sync wait per instruction ---

class _TileContext(TileContext):
    def _drain_and_barrier(self, tick_clock, wait_clock):
        drain_inst = self.nc.sync.drain()
        wait_clock.add_sem_waits(
            drain_inst.ins, ScopedClock({None: tick_clock.global_clock})
        )
        si = drain_inst.ins.sync_info
        if si is not None and len(si.on_wait) > 1:
            waits = list(si.on_wait)
            drain_inst.ins.sync_info = mybir.SyncInfo(on_wait=waits[:1], on_update=[])
            for i in range(1, len(waits)):
                extra = self.nc.sync.drain()
                extra.ins.sync_info = mybir.SyncInfo(on_wait=waits[i:i + 1], on_update=[])
        self.nc.all_engine_barrier()
        assert self.sems is not None
        popped = self.nc._tile_sem_poison_stack.pop()
        assert popped is self._sem_poison
        self.nc.clear_and_free_semaphores(list(self.sems.allocated().values()))
        self.nc.all_engine_barrier()


def _split_waits(nc, max_waits=1):
    n_new = 0
    for f in nc.m.functions:
        for bb in f.blocks:
            insts = list(bb.instructions)
            out = []
            changed = False
            for inst in insts:
                si = inst.sync_info
                if si is not None and len(si.on_wait) > max_waits:
                    changed = True
                    waits = list(si.on_wait)
                    extra, keep = waits[:-max_waits], waits[-max_waits:]
                    for w in extra:
                        n_new += 1
                        nop = mybir.InstNoOp(
                            name=f"I-wsplit-{n_new}", engine=inst.engine, ins=[], outs=[]
                        )
                        nop.sync_info = mybir.SyncInfo(on_wait=[w], on_update=[])
                        out.append(nop)
                    inst.sync_info = mybir.SyncInfo(
                        on_wait=keep, on_update=list(si.on_update)
                    )
                out.append(inst)
            if changed:
                bb.instructions = out
    return n_new


def build_kernel():
    nc = bass.Bass()

    qT = nc.dram_tensor("qT", [D, S], F16, kind="ExternalInput")
    kT = nc.dram_tensor("kT", [D, S], F16, kind="ExternalInput")
    vT = nc.dram_tensor("vT", [D, S], BF16, kind="ExternalInput")
    wq = nc.dram_tensor("wq", [D, HPC * DK], F16, kind="ExternalInput")
    wk = nc.dram_tensor("wk", [D, HPC * DK], F16, kind="ExternalInput")
    wv = nc.dram_tensor("wv", [D, HPC * DV], BF16, kind="ExternalInput")
    wfc = nc.dram_tensor("wfc", [HPC * DV, D], BF16, kind="ExternalInput")
    rvv = nc.dram_tensor("rvv", [1, S], F32R, kind="ExternalInput")   # 1.0 valid / 0.0
    cmv = nc.dram_tensor("cmv", [1, S], F32R, kind="ExternalInput")   # 0.0 valid / -1e30
    onesb = nc.dram_tensor("onesb", [128, SJT * HPC], BF16, kind="ExternalInput")
    onesf = nc.dram_tensor("onesf", [1, 128], F32R, kind="ExternalInput")
    attn_t = nc.dram_tensor("attn_t", [HPC, S, S], BF16, kind="ExternalOutput")
    out_p = nc.dram_tensor("out_p", [S, D], BF16, kind="ExternalOutput")

    KT = D // 128   # 8 contraction tiles for projections
    JW = 256        # projection j/i window width (1 MiB stream DMAs)

    with _TileContext(nc) as tc:
        with tc.tile_pool(name="persist", bufs=1) as pp:
            # persistent SBUF
            qaug = [pp.tile([DK + 1, S], F32R, name=f"qaug{h}", tag=f"qaug{h}") for h in range(HPC)]
            kaug = [pp.tile([DK + 1, S], F32R, name=f"kaug{h}", tag=f"kaug{h}") for h in range(HPC)]
            vall = pp.tile([128, SJT, HPC * (DV + 1)], BF16, name="vall", tag="vall")
            ot = [pp.tile([128, S], BF16, name=f"ot{t}", tag=f"ot{t}") for t in range(2)]
            cones = pp.tile([1, 128], F32R, name="cones", tag="cones")
            wfc_sb = pp.tile([128, 2, D], BF16, name="wfc_sb", tag="wfc_sb")

            nc.sync.dma_start(out=cones[:], in_=onesf[:])
            nc.sync.dma_start(out=wfc_sb[:], in_=wfc.rearrange("(t p) m -> p t m", p=128))
            # augmented mask rows
            for h in range(HPC):
                nc.sync.dma_start(out=qaug[h][DK:DK + 1, :], in_=rvv[:])
                nc.sync.dma_start(out=kaug[h][DK:DK + 1, :], in_=cmv[:])
            # ones columns of Vaug (col DV of each head block)
            va4 = vall.rearrange("p s (h c) -> p s h c", h=HPC, c=DV + 1)
            nc.sync.dma_start(
                out=va4[:, :, :, DV:DV + 1],
                in_=onesb.rearrange("p (s h) -> p s h", h=HPC).unsqueeze(3),
            )

            # ---- phase 1: all projections (V, Q, K interleaved by the scheduler) ----
            with tc.tile_pool(name="wp", bufs=1) as wp, \
                 tc.tile_pool(name="vstream", bufs=2) as vsp, \
                 tc.tile_pool(name="qkstream", bufs=2) as qksp, \
                 tc.tile_pool(name="psv", bufs=2, space="PSUM") as psvp, \
                 tc.tile_pool(name="psqk", bufs=1, space="PSUM") as psqkp:
                wv_sb = wp.tile([128, KT, HPC * DV], BF16, name="wv_sb", tag="wv")
                wq_sb = wp.tile([128, KT, HPC * DK], F16, name="wq_sb", tag="wq")
                wk_sb = wp.tile([128, KT, HPC * DK], F16, name="wk_sb", tag="wk")
                nc.sync.dma_start(out=wv_sb[:], in_=wv.rearrange("(t p) m -> p t m", p=128))
                nc.sync.dma_start(out=wq_sb[:], in_=wq.rearrange("(t p) m -> p t m", p=128))
                nc.sync.dma_start(out=wk_sb[:], in_=wk.rearrange("(t p) m -> p t m", p=128))
                qT3 = qT.rearrange("(t p) s -> p t s", p=128)
                kT3 = kT.rearrange("(t p) s -> p t s", p=128)
                vT3 = vT.rearrange("(t p) s -> p t s", p=128)

                # V and Q/K projections interleaved (QK uses 512-wide windows,
                # V two 256-wide sub-windows per step) so all streams flow
                for w in range(4):
                    # Q/K window (512 wide)
                    qt_sb = qksp.tile([128, KT, 512], F16, name="qt", tag="qt")
                    kt_sb = qksp.tile([128, KT, 512], F16, name="kt", tag="kt")
                    nc.sync.dma_start(out=qt_sb[:], in_=qT3[:, :, w * 512:(w + 1) * 512])
                    nc.sync.dma_start(out=kt_sb[:], in_=kT3[:, :, w * 512:(w + 1) * 512])
                    psq = [psqkp.tile([128, 512], F32, name=f"psq{pr}", tag=f"psq{pr}")
                           for pr in range(2)]
                    psk = [psqkp.tile([128, 512], F32, name=f"psk{pr}", tag=f"psk{pr}")
                           for pr in range(2)]
                    for t in range(KT):
                        for pr in range(2):
                            nc.tensor.matmul(
                                psq[pr][:], wq_sb[:, t, pr * 128:(pr + 1) * 128],
                                qt_sb[:, t, :],
                                start=(t == 0), stop=(t == KT - 1),
                            )
                            nc.tensor.matmul(
                                psk[pr][:], wk_sb[:, t, pr * 128:(pr + 1) * 128],
                                kt_sb[:, t, :],
                                start=(t == 0), stop=(t == KT - 1),
                            )
                    for pr in range(2):
                        for hh in range(2):
                            h = 2 * pr + hh
                            sl = slice(hh * DK, (hh + 1) * DK)
                            nc.vector.tensor_copy(
                                qaug[h][0:DK, w * 512:(w + 1) * 512], psq[pr][sl, :]
                            )
                            nc.vector.tensor_copy(
                                kaug[h][0:DK, w * 512:(w + 1) * 512], psk[pr][sl, :]
                            )
                    # V: two 256-wide sub-windows
                    for jh in (2 * w, 2 * w + 1):
                        vt_sb = vsp.tile([128, KT, JW], BF16, name="vt", tag="vt")
                        nc.sync.dma_start(out=vt_sb[:], in_=vT3[:, :, jh * JW:(jh + 1) * JW])
                        psv = [psvp.tile([128, HPC * DV], F32, name=f"psv{jj}", tag=f"psv{jj}")
                               for jj in range(JW // 128)]
                        for t in range(KT):
                            for jj in range(JW // 128):
                                nc.tensor.matmul(
                                    psv[jj][:],
                                    vt_sb[:, t, jj * 128:(jj + 1) * 128],
                                    wv_sb[:, t, :],
                                    start=(t == 0), stop=(t == KT - 1),
                                )
                        for jj in range(JW // 128):
                            s = jh * (JW // 128) + jj
                            for h in range(HPC):
                                if h % 2 == 0:
                                    nc.scalar.activation(
                                        va4[:, s, h, 0:DV], psv[jj][:, h * DV:(h + 1) * DV],
                                        AF.Copy
                                    )
                                else:
                                    nc.vector.tensor_copy(
                                        va4[:, s, h, 0:DV], psv[jj][:, h * DV:(h + 1) * DV]
                                    )

            # ---- phase 2: attention chunk loop + folded output projection ----
            with tc.tile_pool(name="ech", bufs=3) as echp, \
                 tc.tile_pool(name="stg", bufs=4) as stgp, \
                 tc.tile_pool(name="rn", bufs=2) as rnp, \
                 tc.tile_pool(name="osb", bufs=3) as osbp, \
                 tc.tile_pool(name="pss", bufs=2, space="PSUM") as pssp, \
                 tc.tile_pool(name="pso", bufs=2, space="PSUM") as psop, \
                 tc.tile_pool(name="psr", bufs=2, space="PSUM") as psrp:
                at3 = [attn_t[h].rearrange("(s p) i -> p s i", p=128) for h in range(HPC)]
                for ic in range(NIC):
                    for h in range(HPC):
                        ech = echp.tile([128, SJT, IC], BF16, name="ech", tag="ech")
                        pso = psop.tile([DV + 1, IC], F32, name="pso", tag="pso")
                        for sp in range(SJT // 2):
                            pss = pssp.tile([128, 2, IC], F32, name="pss", tag="pss")
                            for u in range(2):
                                s = 2 * sp + u
                                nc.tensor.matmul(
                                    pss[:, u, :],
                                    kaug[h][:, s * 128:(s + 1) * 128],
                                    qaug[h][:, ic * IC:(ic + 1) * IC],
                                    start=True, stop=True,
                                )
                            nc.scalar.activation(
                                ech[:, 2 * sp:2 * sp + 2, :], pss[:], AF.Exp
                            )
                            for u in range(2):
                                s = 2 * sp + u
                                nc.tensor.matmul(
                                    pso[:],
                                    vall[:, s, h * (DV + 1):(h + 1) * (DV + 1)],
                                    ech[:, s, :],
                                    start=(s == 0), stop=(s == SJT - 1),
                                )
                        # 1/r as exp(-ln r): Ln on ACT, f32r broadcast matmul,
                        # Exp(scale=-1) on ACT (DVE reciprocal is 3.4us serial
                        # latency in the critical chain -- measured worse)
                        lnr = rnp.tile([1, IC], F32R, name="lnr", tag="lnr")
                        with nc.allow_low_precision(reason="float32r bits == float32"):
                            nc.scalar.activation(lnr[:], pso[DV:DV + 1, :], AF.Ln)
                        psr = psrp.tile([128, IC], F32, name="psr", tag="spsum")
                        nc.tensor.matmul(psr[:], cones[:], lnr[:], start=True, stop=True)
                        rsb = rnp.tile([128, IC], BF16, name="rsb", tag="rsb")
                        nc.scalar.activation(rsb[:], psr[:], AF.Exp, scale=-1.0)
                        # normalized O^T into FC operand tile
                        pr, hh = divmod(h, 2)
                        nc.vector.tensor_tensor(
                            out=ot[pr][hh * DV:(hh + 1) * DV, ic * IC:(ic + 1) * IC],
                            in0=pso[0:DV, :], in1=rsb[0:DV, :], op=MULT,
                        )
                        # normalize E into bf16 staging (releases ech quickly;
                        # DVE 2x mode since all APs are bf16), DMA per half
                        for g in range(2):
                            stg = stgp.tile([128, SJT // 2, IC], BF16, name="stg", tag="stg")
                            nc.vector.tensor_tensor(
                                out=stg[:],
                                in0=ech[:, g * (SJT // 2):(g + 1) * (SJT // 2), :],
                                in1=rsb.unsqueeze(1).broadcast_to([128, SJT // 2, IC]),
                                op=MULT,
                            )
                            nc.sync.dma_start(
                                out=at3[h][:, g * (SJT // 2):(g + 1) * (SJT // 2),
                                           ic * IC:(ic + 1) * IC],
                                in_=stg[:],
                            )
                    # folded FC for this ic window (all heads of this window done)
                    for itl in range(IC // 128):
                        it = ic * (IC // 128) + itl
                        for dc in range(2):
                            psf = psrp.tile([128, 512], F32, name="psf", tag="spsum")
                            for t in range(2):
                                nc.tensor.matmul(
                                    psf[:],
                                    ot[t][:, it * 128:(it + 1) * 128],
                                    wfc_sb[:, t, dc * 512:(dc + 1) * 512],
                                    start=(t == 0), stop=(t == 1),
                                )
                            osb = osbp.tile([128, 512], BF16, name="osb", tag="osb")
                            nc.vector.tensor_copy(osb[:], psf[:])
                            nc.sync.dma_start(
                                out=out_p[it * 128:(it + 1) * 128, dc * 512:(dc + 1) * 512],
                                in_=osb[:],
                            )

    _split_waits(nc)
    return nc


_NC_CACHE = None


def _get_nc():
    global _NC_CACHE
    if _NC_CACHE is None:
        _NC_CACHE = build_kernel()
    return _NC_CACHE


def _make_in_maps(q, k, v, mask, Wq, Wk, Wv, Wfc):
    q = np.asarray(q, np.float32)
    k = np.asarray(k, np.float32)
    v = np.asarray(v, np.float32)
    mask = np.asarray(mask)
    Wq = np.asarray(Wq, np.float32)
    Wk = np.asarray(Wk, np.float32)
    Wv = np.asarray(Wv, np.float32)
    Wfc = np.asarray(Wfc, np.float32)

    onesb = np.ones((128, SJT * HPC), ml_dtypes.bfloat16)
    onesf = np.ones((1, 128), np.float32)
    per_batch = []
    for b in range(B):
        valid = (mask[b] != 0)
        rv = valid.astype(np.float32)
        cm = np.where(valid, 0.0, NEG).astype(np.float32)
        qTm = np.ascontiguousarray((q[b] * (rv * SCALE)[:, None]).T).astype(np.float16)
        kTb = np.ascontiguousarray(k[b].T).astype(np.float16)
        vTb = np.ascontiguousarray(v[b].T).astype(ml_dtypes.bfloat16)
        per_batch.append((qTm, kTb, vTb, rv.reshape(1, S), cm.reshape(1, S)))

    in_maps = []
    for c in range(NCORES):
        b, hg = divmod(c, NCORES // B)
        qTm, kTb, vTb, rv, cm = per_batch[b]
        cols = slice(hg * HPC * DK, (hg + 1) * HPC * DK)
        in_maps.append({
            "qT": qTm, "kT": kTb, "vT": vTb,
            "wq": np.ascontiguousarray(Wq[:, cols]).astype(np.float16),
            "wk": np.ascontiguousarray(Wk[:, cols]).astype(np.float16),
            "wv": np.ascontiguousarray(Wv[:, cols]).astype(ml_dtypes.bfloat16),
            "wfc": np.ascontiguousarray(Wfc[cols, :]).astype(ml_dtypes.bfloat16),
            "rvv": rv, "cmv": cm, "onesb": onesb, "onesf": onesf,
        })
    return in_maps


def run(inputs, trace=False):
    nc = _get_nc()
    in_maps = _make_in_maps(**inputs)
    res = run_bass_kernel_spmd(nc, in_maps, list(range(NCORES)), trace=trace)

    out = np.zeros((B, S, H * DV), np.float32)
    attn = np.empty((B, H, S, S), np.float32)
    for c in range(NCORES):
        b, hg = divmod(c, NCORES // B)
        out[b] += np.asarray(res.results[c]["out_p"], np.float32)
        at = np.asarray(res.results[c]["attn_t"], np.float32)  # [HPC, S(j), S(i)]
        for hi in range(HPC):
            attn[b, hg * HPC + hi] = at[hi].T
    return (out, attn), res


def kernel(**inputs):
    (out, attn), _ = run(inputs, trace=False)
    return out, attn
